# revision 1
# baseline (speedup 1.0000x reference)
"""Trainium2 Bass kernel for a 2-layer 2-relation heterogeneous GCN with mean-pool head.

Sharding: destination nodes (and their incident edges) are sharded across 8
NeuronCores; the full feature table lives in each core's DRAM (layer-0 table is
the input x, the layer-1 table is assembled with an on-device AllGather).  The
small [128,128] weights are replicated.  Mean-pool partial sums are computed
per-core and summed on the host (the unshard step).

Per (output-tile, relation) on device:
  - dma_gather (4 SWDGE queues, int16 indices over two 25000-row table halves)
    pulls the source rows of all incident edges into SBUF, 128 edges/chunk.
  - DVE builds a norm-weighted selection matrix SelT[e,k] = (dstloc_e==k)*w_e.
  - PE accumulates aggT[din, node] = sum_chunks Msg_chunk^T-style matmuls.
  - PE applies W[l,r]; relu/bias on ACT/DVE; layer-1 fuses h2@lin_w and the
    per-graph mean-pool segment matmul into the same pass.
"""

import ml_dtypes
import numpy as np

import concourse.bacc as bacc
import concourse.bass as bass
import concourse.mybir as mybir
import concourse.tile as tile
from concourse.bass_utils import run_bass_kernel_spmd

P = 128
NCORES = 8
EDGE_DT = "f32"  # exact fp32 edge pipeline ("bf16" variant kept for experiments)

# Full-size problem constants (from the reference setup).
FULL = dict(N=50000, E=800000, R=2, L=2, D=128, G=64, C=8)


def _ceil_div(a, b):
    return -(-a // b)


def _prep(x, W, b, lin_w, lin_b, edge_index, batch, sizes):
    """Host-side index/normalization prep.  Returns (meta, in_maps)."""
    N, R, L, D, G, C = (sizes[k] for k in ("N", "R", "L", "D", "G", "C"))
    NS = N // NCORES
    HALF = N // 2
    TILES = _ceil_div(NS, P)

    ei = np.asarray(edge_index, dtype=np.int64)
    batch_np = np.asarray(batch, dtype=np.int64)
    x = np.ascontiguousarray(np.asarray(x, dtype=np.float32))
    W = np.ascontiguousarray(np.asarray(W, dtype=np.float32))
    b = np.asarray(b, dtype=np.float32)
    lin_w = np.ascontiguousarray(np.asarray(lin_w, dtype=np.float32))
    lin_b = np.asarray(lin_b, dtype=np.float32)

    # Per-relation edges with symmetric normalization + self loops.
    per_rel = []
    for r in range(R):
        src = ei[r, 0]
        dst = ei[r, 1]
        deg = np.bincount(dst, minlength=N).astype(np.float32) + 1.0
        isd = (1.0 / np.sqrt(deg)).astype(np.float32)
        w_e = isd[src] * isd[dst]
        s_all = np.concatenate([src, np.arange(N, dtype=np.int64)])
        d_all = np.concatenate([dst, np.arange(N, dtype=np.int64)])
        w_all = np.concatenate([w_e, (isd * isd).astype(np.float32)])
        per_rel.append((s_all, d_all, w_all))

    # Group edges by (core, tile, half); compute per-(r,t,h) chunk counts as the
    # max over cores so every core shares one program structure.
    grouped = [[None] * NCORES for _ in range(R)]
    cnts = np.zeros((R, NCORES, TILES, 2), dtype=np.int64)
    for r in range(R):
        s_all, d_all, w_all = per_rel[r]
        core = d_all // NS
        for c in range(NCORES):
            m = core == c
            s = s_all[m]
            d = d_all[m] - c * NS
            w = w_all[m]
            t = d // P
            h = s // HALF
            key = (t * 2 + h).astype(np.int64)
            order = np.argsort(key, kind="stable")
            s, d, w, key = s[order], d[order], w[order], key[order]
            cnt = np.bincount(key, minlength=TILES * 2).reshape(TILES, 2)
            cnts[r, c] = cnt
            grouped[r][c] = (s, d, w, cnt)

    # chunks per (r, t, h): uniform across cores
    nch = np.maximum(_ceil_div(cnts.max(axis=1), P), 1)  # [R, TILES, 2]
    F_rt = nch.sum(axis=2)  # [R, TILES] chunks per (r, t)
    TOTF = int(F_rt.sum())

    # free-dim offsets per (r, t) into the concatenated arrays
    foff = np.zeros((R, TILES), dtype=np.int64)
    acc = 0
    for r in range(R):
        for t in range(TILES):
            foff[r, t] = acc
            acc += int(F_rt[r, t])

    # Per-core packed arrays.
    in_maps = []
    b_sum = b.sum(axis=1)  # [L, D]
    counts = np.bincount(batch_np, minlength=G).astype(np.float32)
    icnt = (1.0 / np.maximum(counts, 1.0)).astype(np.float32)[:, None]  # [G,1]
    iota = np.tile(np.arange(P, dtype=np.float32)[None, :], (P, 1))  # [P,P] replicated

    for c in range(NCORES):
        idx16 = np.zeros((P, TOTF * 8), dtype=np.int16)
        dlw = np.zeros((P, 2, TOTF), dtype=np.float32)
        for r in range(R):
            s, d, w, cnt = grouped[r][c]
            # start of each (t,h) group within this core's sorted edges
            gstart = np.concatenate([[0], np.cumsum(cnt.ravel())])[:-1].reshape(
                TILES, 2
            )
            for t in range(TILES):
                fo = int(foff[r, t])
                ch_off = 0
                for h in range(2):
                    k = int(nch[r, t, h])
                    n_real = int(cnt[t, h])
                    g0 = int(gstart[t, h])
                    sl = np.zeros(k * P, dtype=np.int64)
                    dl = np.zeros(k * P, dtype=np.int64)
                    wl = np.zeros(k * P, dtype=np.float32)
                    sl[:n_real] = s[g0 : g0 + n_real] % HALF
                    dl[:n_real] = d[g0 : g0 + n_real] % P
                    wl[:n_real] = w[g0 : g0 + n_real]
                    # idx16 wrapped: idx i -> [i%16, i//16], replicated x8
                    iw = sl.astype(np.int16).reshape(k * 8, 16).T  # [16, k*8]
                    col0 = (fo + ch_off) * 8
                    idx16[:, col0 : col0 + k * 8] = np.tile(iw, (8, 1))
                    # dloc/w: edge e=j*128+p -> [p, j]
                    dlw[:, 0, fo + ch_off : fo + ch_off + k] = (
                        dl.astype(np.float32).reshape(k, P).T
                    )
                    dlw[:, 1, fo + ch_off : fo + ch_off + k] = wl.reshape(k, P).T
                    ch_off += k

        bl = np.full(TILES * P, -1.0, dtype=np.float32)
        bl[:NS] = batch_np[c * NS : (c + 1) * NS].astype(np.float32)
        bloc = bl.reshape(TILES, P).T.copy()  # [P, TILES]

        in_maps.append(
            {
                "x": x,
                "xh": x.astype(ml_dtypes.bfloat16) if EDGE_DT == "bf16" else np.zeros((1, 1), np.float32),
                "Wt": W,
                "idx16": idx16,
                "dlw": dlw,
                "bloc": bloc,
                "icnt": icnt,
                "iota": iota,
                "linw": lin_w,
                "b0row": np.tile(b_sum[0][None, :], (P, 1)).copy(),
                "b1col": b_sum[1][:, None].copy(),
            }
        )

    meta = dict(
        N=N,
        NS=NS,
        HALF=HALF,
        TILES=TILES,
        R=R,
        D=D,
        G=G,
        C=C,
        TOTF=TOTF,
        nch=nch,
        F_rt=F_rt,
        foff=foff,
        has_b=bool(np.abs(b).max() > 0.0),
        edge_dt=EDGE_DT,
        lin_b=lin_b,
    )
    return meta, in_maps


def _build(meta):
    N = meta["N"]
    NS = meta["NS"]
    HALF = meta["HALF"]
    TILES = meta["TILES"]
    R = meta["R"]
    D = meta["D"]
    G = meta["G"]
    C = meta["C"]
    TOTF = meta["TOTF"]
    nch = meta["nch"]
    F_rt = meta["F_rt"]
    foff = meta["foff"]
    has_b = meta["has_b"]
    f32 = mybir.dt.float32
    bf16 = mybir.dt.bfloat16
    edt = f32 if meta["edge_dt"] == "f32" else bf16

    nc = bacc.Bacc(
        "TRN2",
        target_bir_lowering=False,
        debug=False,
        num_devices=NCORES,
        num_swdge_queues=4,
        dynamic_dma_scratch_size=49152,
    )
    x_ap = nc.dram_tensor("x", [N, D], f32, kind="ExternalInput").ap()
    xh_shape = [N, D] if meta["edge_dt"] == "bf16" else [1, 1]
    xh_dt = bf16 if meta["edge_dt"] == "bf16" else f32
    xh_ap = nc.dram_tensor("xh", xh_shape, xh_dt, kind="ExternalInput").ap()
    Wt = nc.dram_tensor("Wt", [2, R, D, D], f32, kind="ExternalInput").ap()
    idx16 = nc.dram_tensor("idx16", [P, TOTF * 8], mybir.dt.int16, kind="ExternalInput").ap()
    dlw = nc.dram_tensor("dlw", [P, 2, TOTF], f32, kind="ExternalInput").ap()
    bloc = nc.dram_tensor("bloc", [P, TILES], f32, kind="ExternalInput").ap()
    icnt = nc.dram_tensor("icnt", [G, 1], f32, kind="ExternalInput").ap()
    iota = nc.dram_tensor("iota", [P, P], f32, kind="ExternalInput").ap()
    linw = nc.dram_tensor("linw", [D, C], f32, kind="ExternalInput").ap()
    b0row = nc.dram_tensor("b0row", [P, D], f32, kind="ExternalInput").ap()
    b1col = nc.dram_tensor("b1col", [D, 1], f32, kind="ExternalInput").ap()
    out_part = nc.dram_tensor("out_part", [G, C], f32, kind="ExternalOutput").ap()

    with tile.TileContext(nc) as tc:
        with (
            tc.tile_pool(name="const", bufs=1) as constp,
            tc.tile_pool(name="dram", bufs=1, space="DRAM") as dramp,
            tc.tile_pool(name="seld", bufs=10) as seldp,
            tc.tile_pool(name="idxp", bufs=10) as idxp,
            tc.tile_pool(name="selp", bufs=5) as selp,
            tc.tile_pool(name="msgp", bufs=10) as msgp,
            tc.tile_pool(name="aggs", bufs=4) as aggsp,
            tc.tile_pool(name="hnp", bufs=4) as hnp,
            tc.tile_pool(name="zp", bufs=2) as zp,
            tc.tile_pool(name="pselp", bufs=2) as pselp,
            tc.tile_pool(name="psagg", bufs=3, space="PSUM") as psagg,
            tc.tile_pool(name="pshn", bufs=2, space="PSUM") as pshn,
            tc.tile_pool(name="psz", bufs=2, space="PSUM") as psz,
            tc.tile_pool(name="pspool", bufs=1, space="PSUM") as pspool,
        ):
            # constants
            w_s = [[constp.tile([D, D], f32, tag=f"w{l}{r}", name=f"w{l}{r}") for r in range(R)] for l in range(2)]
            for l in range(2):
                for r in range(R):
                    nc.sync.dma_start(out=w_s[l][r][:], in_=Wt[l, r])
            linw_s = constp.tile([D, C], f32, tag="linw")
            nc.sync.dma_start(out=linw_s[:], in_=linw[:])
            iota_s = constp.tile([P, P], f32, tag="iota")
            nc.sync.dma_start(out=iota_s[:], in_=iota[:])
            bloc_s = constp.tile([P, TILES], f32, tag="bloc")
            nc.sync.dma_start(out=bloc_s[:], in_=bloc[:])
            icnt_s = constp.tile([G, 1], f32, tag="icnt")
            nc.sync.dma_start(out=icnt_s[:], in_=icnt[:])
            b0_s = constp.tile([P, D], f32, tag="b0")
            nc.sync.dma_start(out=b0_s[:], in_=b0row[:])
            b1_s = constp.tile([D, 1], f32, tag="b1")
            nc.sync.dma_start(out=b1_s[:], in_=b1col[:])

            h1full = dramp.tile([N, D], edt)
            AGC = min(8, TILES)
            agb = sorted({((q + 1) * (TILES - 1)) // (AGC - 1) for q in range(AGC - 1)} | {TILES}) if AGC > 1 else [TILES]
            AGC = len(agb)
            ag_rows = []
            _lo = 0
            for q in range(AGC):
                hi = min(agb[q] * P, NS)
                ag_rows.append((_lo, hi))
                _lo = hi
            h1own_q = [
                dramp.tile([hi - lo, D], edt, name=f"h1own{q}")
                for q, (lo, hi) in enumerate(ag_rows)
            ]
            h1ag = [
                dramp.tile([NCORES * (hi - lo), D], edt, name=f"h1ag{q}")
                for q, (lo, hi) in enumerate(ag_rows)
            ]
            pool_ps = pspool.tile([G, C], f32)

            ag_bounds = agb

            ag_done = [False] * AGC
            redis_done = [False] * AGC

            def emit_ag(q):
                lo, hi = ag_rows[q]
                nc.gpsimd.collective_compute(
                    "AllGather",
                    mybir.AluOpType.bypass,
                    replica_groups=[list(range(NCORES))],
                    ins=[h1own_q[q][:].opt()],
                    outs=[h1ag[q][:].opt()],
                )

            def emit_redis(q):
                lo, hi = ag_rows[q]
                nr = hi - lo
                for c in range(NCORES):
                    nc.sync.dma_start(
                        out=h1full[c * NS + lo : c * NS + hi, :],
                        in_=h1ag[q][c * nr : (c + 1) * nr, :],
                    )

            def do_layer(l, table):
                for t in range(TILES):
                    rows = min(P, NS - t * P)
                    agg_sb = []
                    for r in range(R):
                        F = int(F_rt[r, t])
                        fo = int(foff[r, t])
                        seld = seldp.tile([P, 2, F], f32, tag="seld")
                        nc.sync.dma_start(out=seld[:], in_=dlw[:, :, fo : fo + F])
                        idxt = idxp.tile([P, F * 8], mybir.dt.int16, tag="idx")
                        nc.sync.dma_start(out=idxt[:], in_=idx16[:, fo * 8 : (fo + F) * 8])
                        sel = selp.tile([P, F, P], edt, tag="sel")
                        nc.vector.tensor_tensor(
                            out=sel[:],
                            in0=seld[:, 0, :].unsqueeze(2).to_broadcast([P, F, P]),
                            in1=iota_s[:, :].unsqueeze(1).to_broadcast([P, F, P]),
                            op=mybir.AluOpType.is_equal,
                        )
                        nc.vector.tensor_tensor(
                            out=sel[:],
                            in0=sel[:],
                            in1=seld[:, 1, :].unsqueeze(2).to_broadcast([P, F, P]),
                            op=mybir.AluOpType.mult,
                        )
                        ks = [int(nch[r, t, 0]), int(nch[r, t, 1])]
                        msgs = []
                        for h in range(2):
                            k = ks[h]
                            msg_h = msgp.tile([P, k, D], edt, tag="msg")
                            nc.gpsimd.dma_gather(
                                out_ap=msg_h[:],
                                in_ap=table[h * HALF : (h + 1) * HALF, :],
                                idxs_ap=idxt[:, (0 if h == 0 else ks[0]) * 8 : (ks[0] + (ks[1] if h else 0)) * 8],
                                num_idxs=k * P,
                                num_idxs_reg=k * P,
                                elem_size=D,
                                queue_num=(2 * r + h + t) % 4,
                                single_packet=False,
                            )
                            msgs.append(msg_h)
                        agg_ps = psagg.tile([D, P], f32, tag="agg")
                        for h in range(2):
                            coff = 0 if h == 0 else ks[0]
                            for j in range(ks[h]):
                                nc.tensor.matmul(
                                    out=agg_ps[:],
                                    lhsT=msgs[h][:, j, :],
                                    rhs=sel[:, coff + j, :],
                                    start=(h == 0 and j == 0),
                                    stop=(h == 1 and j == ks[1] - 1),
                                )
                        a_s = aggsp.tile([D, P], f32, tag="aggs")
                        nc.vector.tensor_copy(out=a_s[:], in_=agg_ps[:])
                        agg_sb.append(a_s)

                    if l == 0:
                        hn_ps = pshn.tile([P, D], f32, tag="hn")
                        for r in range(R):
                            nc.tensor.matmul(
                                out=hn_ps[:],
                                lhsT=agg_sb[r][:],
                                rhs=w_s[0][r][:],
                                start=(r == 0),
                                stop=(r == R - 1),
                            )
                        hn = hnp.tile([P, D], edt, tag="hnsb")
                        if has_b:
                            hb = hnp.tile([P, D], f32, tag="hbias")
                            nc.vector.tensor_tensor(
                                out=hb[:], in0=hn_ps[:], in1=b0_s[:],
                                op=mybir.AluOpType.add,
                            )
                            nc.scalar.activation(
                                out=hn[:], in_=hb[:], func=mybir.ActivationFunctionType.Relu
                            )
                        else:
                            nc.scalar.activation(
                                out=hn[:], in_=hn_ps[:], func=mybir.ActivationFunctionType.Relu
                            )
                        qi = next(i for i, b in enumerate(ag_bounds) if t < b)
                        q_lo = ag_rows[qi][0]
                        nc.sync.dma_start(
                            out=h1own_q[qi][t * P - q_lo : t * P - q_lo + rows, :],
                            in_=hn[:rows, :],
                        )
                        for _q, _b in enumerate(ag_bounds):
                            if t + 1 == _b + 2 and not ag_done[_q]:
                                emit_ag(_q)
                                ag_done[_q] = True
                            if t + 1 == _b + 4 and not redis_done[_q]:
                                emit_redis(_q)
                                redis_done[_q] = True
                    else:
                        h2_ps = pshn.tile([D, P], f32, tag="hn")
                        for r in range(R):
                            nc.tensor.matmul(
                                out=h2_ps[:],
                                lhsT=w_s[1][r][:],
                                rhs=agg_sb[r][:],
                                start=(r == 0),
                                stop=(r == R - 1),
                            )
                        h2t = hnp.tile([D, P], f32, tag="hnsb")
                        if has_b:
                            nc.scalar.activation(
                                out=h2t[:],
                                in_=h2_ps[:],
                                func=mybir.ActivationFunctionType.Copy,
                                bias=b1_s[:, :1],
                            )
                        else:
                            nc.vector.tensor_copy(out=h2t[:], in_=h2_ps[:])
                        z_ps = psz.tile([P, C], f32, tag="z")
                        nc.tensor.matmul(
                            out=z_ps[:], lhsT=h2t[:], rhs=linw_s[:], start=True, stop=True
                        )
                        z_s = zp.tile([P, C], f32, tag="zs")
                        nc.vector.tensor_copy(out=z_s[:], in_=z_ps[:])
                        psel = pselp.tile([P, G], f32, tag="psel")
                        nc.vector.tensor_tensor(
                            out=psel[:],
                            in0=bloc_s[:, t : t + 1].to_broadcast([P, G]),
                            in1=iota_s[:, :G],
                            op=mybir.AluOpType.is_equal,
                        )
                        nc.tensor.matmul(
                            out=pool_ps[:],
                            lhsT=psel[:],
                            rhs=z_s[:],
                            start=(t == 0),
                            stop=(t == TILES - 1),
                        )

            do_layer(0, x_ap if meta["edge_dt"] == "f32" else xh_ap)
            for q in range(AGC):
                if not ag_done[q]:
                    emit_ag(q)
                    ag_done[q] = True
            for q in range(AGC):
                if not redis_done[q]:
                    emit_redis(q)
                    redis_done[q] = True
            do_layer(1, h1full[:])

            pool_s = zp.tile([G, C], f32, tag="pool")
            nc.vector.tensor_copy(out=pool_s[:], in_=pool_ps[:])
            nc.vector.tensor_scalar_mul(out=pool_s[:], in0=pool_s[:], scalar1=icnt_s[:, :1])
            nc.sync.dma_start(out=out_part[:], in_=pool_s[:])

    nc.compile()
    return nc


_CACHE = {}


def _run(x, W, b, lin_w, lin_b, edge_index, batch, sizes, trace=False):
    meta, in_maps = _prep(x, W, b, lin_w, lin_b, edge_index, batch, sizes)
    key = (sizes["N"], meta["TOTF"], tuple(meta["nch"].ravel().tolist()), meta["has_b"])
    nc = _CACHE.get(key)
    if nc is None:
        nc = _build(meta)
        _CACHE[key] = nc
    res = run_bass_kernel_spmd(
        nc, in_maps, core_ids=list(range(NCORES)), trace=trace
    )
    parts = [res.results[c]["out_part"] for c in range(NCORES)]
    out = np.sum(parts, axis=0) + np.asarray(lin_b, dtype=np.float32)[None, :]
    return out.astype(np.float32), res


def kernel(x, W, b, lin_w, lin_b, edge_index, batch):
    out, _ = _run(x, W, b, lin_w, lin_b, edge_index, batch, FULL)
    return out



# revision 17
# speedup vs baseline: 1.5487x; 1.5487x over previous
"""Trainium2 Bass kernel for a 2-layer 2-relation heterogeneous GCN with mean-pool head.

Sharding: destination nodes (and their incident edges) are sharded across 8
NeuronCores; the full feature table lives in each core's DRAM.  The small
[128,128] weights are replicated.  Mean-pool partial sums are computed
per-core and summed on the host (the unshard step).

v2 design (vs the fp32 per-tile-gather baseline):
  - bf16 edge pipeline end to end: gathered rows, selection matrices, and PE
    matmuls are bf16 (fp32 PSUM accumulate), halving HBM gather traffic and
    running the PE at full rate.
  - Symmetric normalization folded out of the per-edge weights entirely:
    per-relation tables are pre-scaled by isd_r[src] (host for layer 0, ACT
    for layer 1) and isd_r[dst] is applied per-partition on the [dst, D]
    side of each relation's W matmul.  The selection matrix is then a plain
    one-hot (single DVE is_equal pass; padding dloc=-1).
  - Banded gathers: one dma_gather per (layer, relation, table-half, band of
    tiles) instead of per (tile, relation, half) — ~26x fewer SWDGE calls,
    which removes the ~1us/call Q7 fixed cost that dominated the baseline.
  - Layer-1 gathers read the AllGather output layout directly (source
    indices permuted host-side), eliminating the redistribute copies.  The
    int16 half-split boundary is aligned to an AllGather chunk boundary so
    half-A gathers only wait on the first two AG chunks.
"""

import ml_dtypes
import numpy as np

import concourse.bacc as bacc
import concourse.bass as bass
import concourse.mybir as mybir
import concourse.tile as tile
from concourse.bass_utils import run_bass_kernel_spmd

P = 128
NCORES = 8

# Full-size problem constants (from the reference setup).
FULL = dict(N=50000, E=800000, R=2, L=2, D=128, G=64, C=8)

BF16 = ml_dtypes.bfloat16


def _ceil_div(a, b):
    return -(-a // b)


def _prep(x, W, b, lin_w, lin_b, edge_index, batch, sizes):
    """Host-side index/normalization prep.  Returns (meta, in_maps)."""
    N, R, L, D, G, C = (sizes[k] for k in ("N", "R", "L", "D", "G", "C"))
    NS = N // NCORES
    TILES = _ceil_div(NS, P)

    ei = np.asarray(edge_index, dtype=np.int64)
    batch_np = np.asarray(batch, dtype=np.int64)
    x = np.ascontiguousarray(np.asarray(x, dtype=np.float32))
    W = np.ascontiguousarray(np.asarray(W, dtype=np.float32))
    b = np.asarray(b, dtype=np.float32)
    lin_w = np.ascontiguousarray(np.asarray(lin_w, dtype=np.float32))
    lin_b = np.asarray(lin_b, dtype=np.float32)

    # AllGather chunks (in tiles): one Shared output tensor per (relation,
    # chunk) — Shared DRAM allows a single writer instruction — and the
    # layer-1 int16 half-split coincides with the chunk boundary.
    AGT = [0, 25, TILES]
    AGC = len(AGT) - 1
    ag_rows = [(min(AGT[q] * P, NS), min(AGT[q + 1] * P, NS)) for q in range(AGC)]
    ag_base = [8 * lo for lo, _ in ag_rows]  # base row of chunk q in h1all layout
    # layer-0 table halves; layer-1 halves are the two AG chunks
    H0 = N // 2
    H1 = ag_base[1]
    assert H1 <= 32768 and (N - H1) <= 32768 and H0 <= 32768

    # position of node g in the h1all (AllGather output) layout
    def pos1_of(g):
        local = g % NS
        core = g // NS
        pos = np.zeros_like(g)
        for q in range(AGC):
            lo, hi = ag_rows[q]
            m = (local >= lo) & (local < hi)
            nr = hi - lo
            pos[m] = ag_base[q] + core[m] * nr + (local[m] - lo)
        return pos

    # Per-relation normalization
    isd = np.zeros((R, N), dtype=np.float32)
    per_rel = []
    for r in range(R):
        src = ei[r, 0]
        dst = ei[r, 1]
        deg = np.bincount(dst, minlength=N).astype(np.float32) + 1.0
        isd[r] = 1.0 / np.sqrt(deg)
        s_all = np.concatenate([src, np.arange(N, dtype=np.int64)])
        d_all = np.concatenate([dst, np.arange(N, dtype=np.int64)])
        per_rel.append((s_all, d_all))

    # Group edges by (layer, relation, core, tile, half); chunk counts are the
    # max over cores so every core shares one program structure.
    # grouped[l][r][c] = (sloc_sorted, dloc_sorted, cnt[t,h])
    grouped = [[[None] * NCORES for _ in range(R)] for _ in range(L)]
    cnts = np.zeros((L, R, NCORES, TILES, 2), dtype=np.int64)
    for r in range(R):
        s_all, d_all = per_rel[r]
        core = d_all // NS
        spos1 = pos1_of(s_all)
        for c in range(NCORES):
            m = core == c
            d = d_all[m] - c * NS
            t = d // P
            for l in range(L):
                sp = (s_all if l == 0 else spos1)[m]
                hb = H0 if l == 0 else H1
                h = (sp >= hb).astype(np.int64)
                key = t * 2 + h
                order = np.argsort(key, kind="stable")
                sp_s, d_s, key_s = sp[order], d[order], key[order]
                cnt = np.bincount(key_s, minlength=TILES * 2).reshape(TILES, 2)
                cnts[l, r, c] = cnt
                grouped[l][r][c] = (sp_s, d_s, cnt)

    # chunks per (l, r, t, h): uniform across cores
    nch = np.maximum(_ceil_div(cnts.max(axis=2), P), 1)  # [L, R, TILES, 2]

    # gather-index layout: per layer, (r, h)-major blocks, tile-order inside.
    # gx_off[l][r][h][t] = chunk offset of tile t within layer l's idx array.
    gx_off = np.zeros((L, R, 2, TILES + 1), dtype=np.int64)
    TOT = [0, 0]
    for l in range(L):
        acc = 0
        for r in range(R):
            for h in range(2):
                for t in range(TILES):
                    gx_off[l, r, h, t] = acc
                    acc += int(nch[l, r, t, h])
                gx_off[l, r, h, TILES] = acc
        TOT[l] = acc

    # seld layout: per layer, (r, t)-major, h0 chunks then h1 chunks.
    sd_off = np.zeros((L, R, TILES), dtype=np.int64)
    for l in range(L):
        acc = 0
        for r in range(R):
            for t in range(TILES):
                sd_off[l, r, t] = acc
                acc += int(nch[l, r, t, 0] + nch[l, r, t, 1])

    # Per-core packed arrays.
    in_maps = []
    counts = np.bincount(batch_np, minlength=G).astype(np.float32)
    icnt = (1.0 / np.maximum(counts, 1.0)).astype(np.float32)[:, None]  # [G,1]
    iota = np.tile(
        np.arange(P, dtype=np.float32).astype(BF16)[None, :], (P, 1)
    )  # [P,P]

    t0 = (x * isd[0][:, None]).astype(BF16)
    t1 = (x * isd[1][:, None]).astype(BF16)
    Wb = W.astype(BF16)
    linwb = lin_w.astype(BF16)
    b0row = np.tile(b.sum(axis=1)[0][None, :], (P, 1)).astype(np.float32)

    for c in range(NCORES):
        idx16 = [np.zeros((P, TOT[l] * 8), dtype=np.int16) for l in range(L)]
        seld = [np.full((P, TOT[l]), -1.0, dtype=BF16) for l in range(L)]
        for l in range(L):
            for r in range(R):
                sp_s, d_s, cnt = grouped[l][r][c]
                gstart = np.concatenate([[0], np.cumsum(cnt.ravel())])[:-1].reshape(
                    TILES, 2
                )
                for t in range(TILES):
                    for h in range(2):
                        k = int(nch[l, r, t, h])
                        n_real = int(cnt[t, h])
                        g0 = int(gstart[t, h])
                        sl = np.zeros(k * P, dtype=np.int64)
                        dl = np.full(k * P, -1.0, dtype=np.float32)
                        sbase = (H0 if l == 0 else H1) if h == 1 else 0
                        sl[:n_real] = sp_s[g0 : g0 + n_real] - sbase
                        dl[:n_real] = (d_s[g0 : g0 + n_real] - t * P).astype(
                            np.float32
                        )
                        # idx16 wrapped: idx i -> [i%16, i//16], replicated x8
                        iw = sl.astype(np.int16).reshape(k * 8, 16).T
                        col0 = int(gx_off[l, r, h, t]) * 8
                        idx16[l][:, col0 : col0 + k * 8] = np.tile(iw, (8, 1))
                        # dloc: edge e=j*128+p -> [p, j]
                        sc = int(sd_off[l, r, t]) + (0 if h == 0 else int(nch[l, r, t, 0]))
                        seld[l][:, sc : sc + k] = (
                            dl.reshape(k, P).T.astype(BF16)
                        )

        bl = np.full(TILES * P, -1.0, dtype=np.float32)
        bl[:NS] = batch_np[c * NS : (c + 1) * NS].astype(np.float32)
        bloc = bl.reshape(TILES, P).T.astype(BF16).copy()  # [P, TILES]

        sdl = np.zeros(TILES * P, dtype=np.float32)
        sdl_src = isd[:, c * NS : (c + 1) * NS]  # [R, NS]
        isddst = np.zeros((R, P, TILES), dtype=np.float32)
        for r in range(R):
            sdl[:NS] = sdl_src[r]
            sdl[NS:] = 0.0
            isddst[r] = sdl.reshape(TILES, P).T

        in_maps.append(
            {
                "t0": t0,
                "t1": t1,
                "Wt": Wb,
                "idx0": idx16[0],
                "idx1": idx16[1],
                "sd0": seld[0],
                "sd1": seld[1],
                "bloc": bloc,
                "icnt": icnt,
                "iota": iota,
                "linw": linwb,
                "isdd0": np.ascontiguousarray(isddst[0]),
                "isdd1": np.ascontiguousarray(isddst[1]),
                "b0row": b0row,
            }
        )

    meta = dict(
        N=N,
        NS=NS,
        TILES=TILES,
        R=R,
        D=D,
        G=G,
        C=C,
        TOT=TOT,
        nch=nch,
        gx_off=gx_off,
        sd_off=sd_off,
        AGT=AGT,
        ag_rows=ag_rows,
        ag_base=ag_base,
        H0=H0,
        H1=H1,
        has_b=bool(np.abs(b).max() > 0.0),
        lin_b=lin_b,
        b_sum=b.sum(axis=1),
        lin_w=lin_w,
    )
    return meta, in_maps


def _build(meta):
    N = meta["N"]
    NS = meta["NS"]
    TILES = meta["TILES"]
    R = meta["R"]
    D = meta["D"]
    G = meta["G"]
    C = meta["C"]
    TOT = meta["TOT"]
    nch = meta["nch"]
    gx_off = meta["gx_off"]
    sd_off = meta["sd_off"]
    AGT = meta["AGT"]
    ag_rows = meta["ag_rows"]
    ag_base = meta["ag_base"]
    H0 = meta["H0"]
    H1 = meta["H1"]
    has_b = meta["has_b"]
    f32 = mybir.dt.float32
    bf16 = mybir.dt.bfloat16
    i16 = mybir.dt.int16
    AGC = len(ag_rows)
    TB = 4  # tiles per gather band

    nc = bacc.Bacc(
        "TRN2",
        target_bir_lowering=False,
        debug=False,
        num_devices=NCORES,
        num_swdge_queues=4,
        dynamic_dma_scratch_size=49152,
    )
    t0_ap = nc.dram_tensor("t0", [N, D], bf16, kind="ExternalInput").ap()
    t1_ap = nc.dram_tensor("t1", [N, D], bf16, kind="ExternalInput").ap()
    tabs0 = [t0_ap, t1_ap]
    Wt = nc.dram_tensor("Wt", [2, R, D, D], bf16, kind="ExternalInput").ap()
    idx0 = nc.dram_tensor("idx0", [P, TOT[0] * 8], i16, kind="ExternalInput").ap()
    idx1 = nc.dram_tensor("idx1", [P, TOT[1] * 8], i16, kind="ExternalInput").ap()
    sd0 = nc.dram_tensor("sd0", [P, TOT[0]], bf16, kind="ExternalInput").ap()
    sd1 = nc.dram_tensor("sd1", [P, TOT[1]], bf16, kind="ExternalInput").ap()
    bloc = nc.dram_tensor("bloc", [P, TILES], bf16, kind="ExternalInput").ap()
    icnt = nc.dram_tensor("icnt", [G, 1], f32, kind="ExternalInput").ap()
    iota = nc.dram_tensor("iota", [P, P], bf16, kind="ExternalInput").ap()
    linw = nc.dram_tensor("linw", [D, C], bf16, kind="ExternalInput").ap()
    isdd0 = nc.dram_tensor("isdd0", [P, TILES], f32, kind="ExternalInput").ap()
    isdd1 = nc.dram_tensor("isdd1", [P, TILES], f32, kind="ExternalInput").ap()
    b0row = nc.dram_tensor("b0row", [P, D], f32, kind="ExternalInput").ap()
    out_part = nc.dram_tensor("out_part", [G, C], f32, kind="ExternalOutput").ap()

    with tile.TileContext(nc) as tc:
        with (
            tc.tile_pool(name="const", bufs=1) as constp,
            tc.tile_pool(name="dram", bufs=1, space="DRAM") as dramp,
            tc.tile_pool(name="msgp", bufs=8) as msgp,
            tc.tile_pool(name="selp", bufs=4) as selp,
            tc.tile_pool(name="aggs", bufs=4) as aggsp,
            tc.tile_pool(name="hsp", bufs=6) as hsp,
            tc.tile_pool(name="h1wp", bufs=4) as h1wp,
            tc.tile_pool(name="h2tp", bufs=4) as h2tp,
            tc.tile_pool(name="zp", bufs=4) as zp,
            tc.tile_pool(name="pselp", bufs=2) as pselp,
            tc.tile_pool(name="psagg", bufs=3, space="PSUM") as psagg,
            tc.tile_pool(name="pshn", bufs=2, space="PSUM") as pshn,
            tc.tile_pool(name="psz", bufs=2, space="PSUM") as psz,
            tc.tile_pool(name="pspool", bufs=1, space="PSUM") as pspool,
        ):
            # constants
            w_s = [
                [constp.tile([D, D], bf16, tag=f"w{l}{r}", name=f"w{l}{r}") for r in range(R)]
                for l in range(2)
            ]
            for l in range(2):
                for r in range(R):
                    nc.sync.dma_start(out=w_s[l][r][:], in_=Wt[l, r])
            linw_s = constp.tile([D, C], bf16, tag="linw", name="linw")
            nc.sync.dma_start(out=linw_s[:], in_=linw[:])
            iota_s = constp.tile([P, P], bf16, tag="iota", name="iota")
            nc.sync.dma_start(out=iota_s[:], in_=iota[:])
            bloc_s = constp.tile([P, TILES], bf16, tag="bloc", name="bloc")
            nc.sync.dma_start(out=bloc_s[:], in_=bloc[:])
            icnt_s = constp.tile([G, 1], f32, tag="icnt", name="icnt")
            nc.sync.dma_start(out=icnt_s[:], in_=icnt[:])
            isdd_s = [constp.tile([P, TILES], f32, tag=f"isdd{r}", name=f"isdd{r}") for r in range(R)]
            nc.sync.dma_start(out=isdd_s[0][:], in_=isdd0[:])
            nc.sync.dma_start(out=isdd_s[1][:], in_=isdd1[:])
            if has_b:
                b0_s = constp.tile([P, D], f32, tag="b0", name="b0")
                nc.sync.dma_start(out=b0_s[:], in_=b0row[:])

            # index/seld tables: one buffer, reused across layers (WAR dep:
            # the layer-1 load waits for the last layer-0 gather read)
            MAXT = max(TOT)
            idx_l0 = constp.tile([P, TOT[0] * 8], i16, tag="idx", name="idx_l0", padded_shape=[P, MAXT * 8])
            nc.sync.dma_start(out=idx_l0[:], in_=idx0[:])
            sd_l0 = constp.tile([P, TOT[0]], bf16, tag="sd", name="sd_l0", padded_shape=[P, MAXT])
            nc.sync.dma_start(out=sd_l0[:], in_=sd0[:])

            # layer-1 tables: AllGather outputs, one Shared tensor per
            # (relation, chunk); chunk == int16 half for layer-1 gathers
            h1sh = [
                [
                    dramp.tile(
                        [8 * (hi - lo), D],
                        bf16,
                        name=f"h1sh{r}_{q}",
                        addr_space="Shared",
                    )
                    for q, (lo, hi) in enumerate(ag_rows)
                ]
                for r in range(R)
            ]

            # AllGather input staging (per relation, per chunk)
            h1own = [
                [
                    dramp.tile([hi - lo, D], bf16, name=f"h1own{r}_{q}")
                    for q, (lo, hi) in enumerate(ag_rows)
                ]
                for r in range(R)
            ]

            pool_ps = pspool.tile([G, C], f32, name="poolps")

            ag_done = [False] * AGC

            def emit_ag(q):
                for r in range(R):
                    nc.gpsimd.collective_compute(
                        "AllGather",
                        mybir.AluOpType.bypass,
                        replica_groups=[list(range(NCORES))],
                        ins=[h1own[r][q][:].opt()],
                        outs=[h1sh[r][q][:].opt()],
                    )

            halves0 = [(0, H0), (H0, N)]

            def do_layer(l, idx_t, sd_t):
                bands = [
                    (tb, min(tb + TB, TILES)) for tb in range(0, TILES, TB)
                ]
                # gather one band ahead
                msg_tiles = {}  # (band_idx, r, h) -> (tile, K)

                def gather_band(bi):
                    tb0, tb1 = bands[bi]
                    for r in range(R):
                        for h in range(2):
                            c0 = int(gx_off[l, r, h, tb0])
                            c1 = int(gx_off[l, r, h, tb1])
                            K = c1 - c0
                            mt = msgp.tile([P, K, D], bf16, tag="msg", name="msg")
                            if l == 0:
                                s0, s1 = halves0[h]
                                src_ap = tabs0[r][s0:s1, :]
                            else:
                                src_ap = h1sh[r][h][:]
                            nc.gpsimd.dma_gather(
                                out_ap=mt[:],
                                in_ap=src_ap,
                                idxs_ap=idx_t[:, c0 * 8 : c1 * 8],
                                num_idxs=K * P,
                                num_idxs_reg=K * P,
                                elem_size=D,
                                queue_num=2 * r + h,
                                single_packet=False,
                            )
                            msg_tiles[(bi, r, h)] = (mt, c0)

                gather_band(0)
                for bi, (tb0, tb1) in enumerate(bands):
                    if bi + 1 < len(bands):
                        gather_band(bi + 1)
                    for t in range(tb0, tb1):
                        rows = min(P, NS - t * P)
                        a_s = []
                        for r in range(R):
                            k0 = int(nch[l, r, t, 0])
                            k1 = int(nch[l, r, t, 1])
                            F = k0 + k1
                            sc = int(sd_off[l, r, t])
                            sel = selp.tile([P, F, P], bf16, tag="sel", name="sel")
                            nc.vector.tensor_tensor(
                                out=sel[:],
                                in0=sd_t[:, sc : sc + F]
                                .unsqueeze(2)
                                .to_broadcast([P, F, P]),
                                in1=iota_s[:, :].unsqueeze(1).to_broadcast([P, F, P]),
                                op=mybir.AluOpType.is_equal,
                            )
                            agg_ps = psagg.tile([D, P], f32, tag="agg", name="agg")
                            for h, k in ((0, k0), (1, k1)):
                                mt, c0 = msg_tiles[(bi, r, h)]
                                jl0 = int(gx_off[l, r, h, t]) - c0
                                for j in range(k):
                                    nc.tensor.matmul(
                                        out=agg_ps[:],
                                        lhsT=mt[:, jl0 + j, :],
                                        rhs=sel[:, (j if h == 0 else k0 + j), :],
                                        start=(h == 0 and j == 0),
                                        stop=(h == 1 and j == k - 1),
                                    )
                            a_sb = aggsp.tile([D, P], bf16, tag="aggs", name="aggs")
                            nc.vector.tensor_copy(out=a_sb[:], in_=agg_ps[:])
                            a_s.append(a_sb)

                        if l == 0:
                            hs = []
                            for r in range(R):
                                hn_ps = pshn.tile([P, D], f32, tag="hn", name="hn")
                                nc.tensor.matmul(
                                    out=hn_ps[:],
                                    lhsT=a_s[r][:],
                                    rhs=w_s[0][r][:],
                                    start=True,
                                    stop=True,
                                )
                                hs_r = hsp.tile([P, D], f32, tag="hs", name="hs")
                                nc.scalar.activation(
                                    out=hs_r[:],
                                    in_=hn_ps[:],
                                    func=mybir.ActivationFunctionType.Copy,
                                    scale=isdd_s[r][:, t : t + 1],
                                )
                                hs.append(hs_r)
                            hsum = hsp.tile([P, D], f32, tag="hsum", name="hsum")
                            nc.vector.tensor_tensor(
                                out=hsum[:],
                                in0=hs[0][:],
                                in1=hs[1][:],
                                op=mybir.AluOpType.add,
                            )
                            if has_b:
                                nc.vector.tensor_tensor(
                                    out=hsum[:],
                                    in0=hsum[:],
                                    in1=b0_s[:],
                                    op=mybir.AluOpType.add,
                                )
                            qi = next(
                                i for i in range(AGC) if t < AGT[i + 1]
                            )
                            lo_q = ag_rows[qi][0]
                            for r in range(R):
                                h1w = h1wp.tile([P, D], bf16, tag="h1w", name="h1w")
                                nc.scalar.activation(
                                    out=h1w[:],
                                    in_=hsum[:],
                                    func=mybir.ActivationFunctionType.Relu,
                                    scale=isdd_s[r][:, t : t + 1],
                                )
                                nc.sync.dma_start(
                                    out=h1own[r][qi][
                                        t * P - lo_q : t * P - lo_q + rows, :
                                    ],
                                    in_=h1w[:rows, :],
                                )
                            for q in range(AGC):
                                if not ag_done[q] and t + 1 > AGT[q + 1]:
                                    emit_ag(q)
                                    ag_done[q] = True
                        else:
                            for r in range(R):
                                h2_ps = pshn.tile([D, P], f32, tag="hn", name="hn2")
                                nc.tensor.matmul(
                                    out=h2_ps[:],
                                    lhsT=w_s[1][r][:],
                                    rhs=a_s[r][:],
                                    start=True,
                                    stop=True,
                                )
                                h2t = h2tp.tile([D, P], bf16, tag="h2t", name="h2t")
                                nc.vector.tensor_copy(out=h2t[:], in_=h2_ps[:])
                                z_ps = psz.tile([P, C], f32, tag="z", name="z")
                                nc.tensor.matmul(
                                    out=z_ps[:],
                                    lhsT=h2t[:],
                                    rhs=linw_s[:],
                                    start=True,
                                    stop=True,
                                )
                                z_s = zp.tile([P, C], bf16, tag="zs", name="zs")
                                nc.scalar.activation(
                                    out=z_s[:],
                                    in_=z_ps[:],
                                    func=mybir.ActivationFunctionType.Copy,
                                    scale=isdd_s[r][:, t : t + 1],
                                )
                                if r == 0:
                                    psel = pselp.tile([P, G], bf16, tag="psel", name="psel")
                                    nc.vector.tensor_tensor(
                                        out=psel[:],
                                        in0=bloc_s[:, t : t + 1].to_broadcast([P, G]),
                                        in1=iota_s[:, :G],
                                        op=mybir.AluOpType.is_equal,
                                    )
                                nc.tensor.matmul(
                                    out=pool_ps[:],
                                    lhsT=psel[:],
                                    rhs=z_s[:],
                                    start=(t == 0 and r == 0),
                                    stop=(t == TILES - 1 and r == R - 1),
                                )

            do_layer(0, idx_l0, sd_l0)
            for q in range(AGC):
                if not ag_done[q]:
                    emit_ag(q)
                    ag_done[q] = True
            # load layer-1 index tables into the same buffers (overlaps AG tail)
            idx_l1 = constp.tile(
                [P, TOT[1] * 8], i16, tag="idx", name="idx_l1", padded_shape=[P, MAXT * 8]
            )
            nc.sync.dma_start(out=idx_l1[:], in_=idx1[:])
            sd_l1 = constp.tile([P, TOT[1]], bf16, tag="sd", name="sd_l1", padded_shape=[P, MAXT])
            nc.sync.dma_start(out=sd_l1[:], in_=sd1[:])
            do_layer(1, idx_l1, sd_l1)

            pool_s = zp.tile([G, C], f32, tag="pool", name="pool")
            nc.vector.tensor_copy(out=pool_s[:], in_=pool_ps[:])
            nc.vector.tensor_scalar_mul(
                out=pool_s[:], in0=pool_s[:], scalar1=icnt_s[:, :1]
            )
            nc.sync.dma_start(out=out_part[:], in_=pool_s[:])

    nc.compile()
    return nc


_CACHE = {}


def _run(x, W, b, lin_w, lin_b, edge_index, batch, sizes, trace=False):
    meta, in_maps = _prep(x, W, b, lin_w, lin_b, edge_index, batch, sizes)
    key = (
        sizes["N"],
        tuple(meta["TOT"]),
        tuple(meta["nch"].ravel().tolist()),
        meta["has_b"],
    )
    nc = _CACHE.get(key)
    if nc is None:
        nc = _build(meta)
        _CACHE[key] = nc
    res = run_bass_kernel_spmd(nc, in_maps, core_ids=list(range(NCORES)), trace=trace)
    parts = [res.results[c]["out_part"] for c in range(NCORES)]
    out = np.sum(parts, axis=0) + np.asarray(lin_b, dtype=np.float32)[None, :]
    if meta["has_b"]:
        # layer-1 per-relation biases pass linearly through the head
        out = out + (meta["b_sum"][1] @ meta["lin_w"])[None, :]
    return out.astype(np.float32), res


def kernel(x, W, b, lin_w, lin_b, edge_index, batch):
    out, _ = _run(x, W, b, lin_w, lin_b, edge_index, batch, FULL)
    return out
